# revision 9
# baseline (speedup 1.0000x reference)
"""Distributed attention kernel for trn2 (8 NeuronCores, SPMD).

Layer: B=1, S=2048, DIM=2048, H=16, HD=128 causal attention with SSMax
(section_log_len) scaling, fp32 reference, rel-err budget 2e-2.

Strategy (sequence-parallel, uniform SPMD graph):
- Query rows are sharded mod-8 (core i owns rows i, i+8, ...) so the causal
  key-block ranges are identical on every core: q-tile 0 (rows < 1024+i)
  needs key blocks 0..7, q-tile 1 needs key blocks 0..15.  Causal masking
  becomes per-core host-prepared mask *data* (multiplicative, exp(mask)).
- K/V rows are sharded contiguously (core i owns rows 256i..256i+256);
  each core projects its K^T/V shard and two AllGathers (fp16 K^T, bf16 V,
  1 MB per rank each) distribute them, overlapped with the Q projection.
- All matmuls run in transposed layouts so no on-device transposes exist:
    kT_h (d,k) = wkT_blk^T @ xkvT_blk          qT_h (d,q) = wqT_blk^T @ xqT_blk
    scoresT (k,q) = kT_blk^T @ qT_blk          [softmax over k = partition axis]
    oT_h (d,q) = v_blk^T @ expT_blk            final (q,o) = attnT_blk^T @ woT_blk
- Softmax without max-subtraction (scores <= ~80, exp fits fp32/bf16 range);
  sums via DVE accumulation + one fp32 ones-matmul partition-reduce; the
  1/sum normalization is applied at PSUM eviction of oT via a rank-1
  matmul broadcast of the reciprocal row.
- Precision: fp16 inputs on the q/k score path (sll amplifies scores ~8x,
  bf16 there fails the 2e-2 gate), bf16 for exp/V, fp16 attnT/wo,
  fp32 PSUM accumulation everywhere.  Host-side numpy sim: max-rel 2.6e-3.
- sll (per-query) is folded into x_q rows on the host; seq_scale/sqrt(HD)
  is folded into wq columns on the host.
"""

import math

import numpy as np
import ml_dtypes

S = 2048
DIM = 2048
H = 16
HD = 128
N_CORES = 8
SQ = S // N_CORES          # 256 query rows per core (mod-8 interleaved)
SKV = S // N_CORES         # 256 kv rows per core (contiguous)
CB = DIM // 128            # 16 contraction blocks
QT = 2                     # q-tiles of 128 per core
KB_PER_T = (8, 16)         # key blocks per q-tile (uniform across cores)
N_MASK = KB_PER_T[0] + KB_PER_T[1]   # 24 mask tiles (t0 kb0..7, t1 kb0..15)

_CACHE = {}
LAST_RESULT = None


def _build_graph():
    import concourse.bass as bass
    import concourse.tile as tile
    from concourse import mybir

    f16 = mybir.dt.float16
    bf16 = mybir.dt.bfloat16
    f32 = mybir.dt.float32

    nc = bass.Bass()
    xqT_e = nc.declare_dram_parameter("xqT", [DIM, SQ], f16, isOutput=False)
    xkvT_e = nc.declare_dram_parameter("xkvT", [DIM, SKV], f16, isOutput=False)
    wqT_e = nc.declare_dram_parameter("wqT", [DIM, DIM], f16, isOutput=False)
    wkT_e = nc.declare_dram_parameter("wkT", [DIM, DIM], f16, isOutput=False)
    wvT_e = nc.declare_dram_parameter("wvT", [DIM, DIM], f16, isOutput=False)
    woT_e = nc.declare_dram_parameter("woT", [DIM, DIM], f16, isOutput=False)
    maskM_e = nc.declare_dram_parameter("maskM", [N_MASK, 128, 128], bf16, isOutput=False)
    out_e = nc.declare_dram_parameter("out", [SQ, DIM], f32, isOutput=True)

    with tile.TileContext(nc) as tc:
        with (
            tc.tile_pool(name="persist", bufs=1) as persist,
            tc.tile_pool(name="dram", bufs=1, space="DRAM") as dram,
        ):
            # ---- resident inputs -------------------------------------------------
            xqT_sb = persist.tile([128, CB, SQ], f16)
            nc.sync.dma_start(out=xqT_sb[:], in_=xqT_e[:].rearrange("(cb p) q -> p cb q", p=128))
            xkvT_sb = persist.tile([128, CB, SKV], f16)
            nc.sync.dma_start(out=xkvT_sb[:], in_=xkvT_e[:].rearrange("(cb p) q -> p cb q", p=128))
            mask_sb = persist.tile([128, N_MASK, 128], bf16)
            nc.sync.dma_start(out=mask_sb[:], in_=maskM_e[:].rearrange("j p r -> p j r"))
            ones_sb = persist.tile([128, 1], f32)
            nc.vector.memset(ones_sb[:], 1.0)
            ones_row = persist.tile([1, 128], f32)
            nc.vector.memset(ones_row[:], 1.0)
            qT_sb = persist.tile([128, H, SQ], f16)
            attnT_sb = persist.tile([128, QT, H, 128], f16)

            # ---- collective bounce buffers --------------------------------------
            cc_in_k = dram.tile([DIM, SKV], f16)
            cc_out_k = dram.tile([N_CORES * DIM, SKV], f16, addr_space="Shared")
            cc_in_v = dram.tile([SKV, DIM], bf16)
            cc_out_v = dram.tile([N_CORES * SKV, DIM], bf16, addr_space="Shared")

            # ---- phase 1: K^T shard projection + AllGather ----------------------
            with (
                tc.tile_pool(name="wk_pool", bufs=3) as wk_pool,
                tc.tile_pool(name="kev_pool", bufs=3) as kev_pool,
                tc.tile_pool(name="pk_psum", bufs=2, space="PSUM") as pk_psum,
            ):
                for h in range(H):
                    wk_h = wk_pool.tile([128, CB, 128], f16, tag="wk")
                    nc.sync.dma_start(
                        out=wk_h[:],
                        in_=wkT_e[:].rearrange("(cb p) (hh d) -> hh p cb d", p=128, hh=H)[h],
                    )
                    ps = pk_psum.tile([128, SKV], f32, tag="pk")
                    for cb in range(CB):
                        nc.tensor.matmul(
                            ps[:], wk_h[:, cb, :], xkvT_sb[:, cb, :],
                            start=(cb == 0), stop=(cb == CB - 1),
                        )
                    kev = kev_pool.tile([128, SKV], f16, tag="kev")
                    nc.vector.tensor_copy(kev[:], ps[:])
                    nc.sync.dma_start(out=cc_in_k[h * 128:(h + 1) * 128, :], in_=kev[:])

                nc.gpsimd.collective_compute(
                    "AllGather", mybir.AluOpType.bypass,
                    ins=[cc_in_k.opt()], outs=[cc_out_k.opt()],
                    replica_groups=[list(range(N_CORES))],
                )

            # ---- phase 2: V shard projection + AllGather ------------------------
            with (
                tc.tile_pool(name="wv_pool", bufs=4) as wv_pool,
                tc.tile_pool(name="vev_pool", bufs=3) as vev_pool,
                tc.tile_pool(name="pv_psum", bufs=2, space="PSUM") as pv_psum,
            ):
                for ng in range(4):
                    ps_s = [pv_psum.tile([128, 512], f32, tag="pv", name=f"pv{ng}_{s}") for s in range(2)]
                    for cb in range(CB):
                        wv_t = wv_pool.tile([128, 512], f16, tag="wv")
                        nc.sync.dma_start(
                            out=wv_t[:],
                            in_=wvT_e[cb * 128:(cb + 1) * 128, ng * 512:(ng + 1) * 512],
                        )
                        for s in range(2):
                            nc.tensor.matmul(
                                ps_s[s][:], xkvT_sb[:, cb, s * 128:(s + 1) * 128], wv_t[:],
                                start=(cb == 0), stop=(cb == CB - 1),
                            )
                    for s in range(2):
                        vev = vev_pool.tile([128, 512], bf16, tag="vev")
                        nc.vector.tensor_copy(vev[:], ps_s[s][:])
                        nc.sync.dma_start(
                            out=cc_in_v[s * 128:(s + 1) * 128, ng * 512:(ng + 1) * 512],
                            in_=vev[:],
                        )

                nc.gpsimd.collective_compute(
                    "AllGather", mybir.AluOpType.bypass,
                    ins=[cc_in_v.opt()], outs=[cc_out_v.opt()],
                    replica_groups=[list(range(N_CORES))],
                )

            # ---- phase 3: Q^T projection (overlaps the AllGathers) --------------
            with (
                tc.tile_pool(name="wq_pool", bufs=3) as wq_pool,
                tc.tile_pool(name="pq_psum", bufs=2, space="PSUM") as pq_psum,
            ):
                for h in range(H):
                    wq_h = wq_pool.tile([128, CB, 128], f16, tag="wq")
                    nc.sync.dma_start(
                        out=wq_h[:],
                        in_=wqT_e[:].rearrange("(cb p) (hh d) -> hh p cb d", p=128, hh=H)[h],
                    )
                    ps = pq_psum.tile([128, SQ], f32, tag="pq")
                    for cb in range(CB):
                        nc.tensor.matmul(
                            ps[:], wq_h[:, cb, :], xqT_sb[:, cb, :],
                            start=(cb == 0), stop=(cb == CB - 1),
                        )
                    nc.vector.tensor_copy(qT_sb[:, h, :], ps[:])

            # ---- phase 4: attention --------------------------------------------
            kt_view = cc_out_k[:].rearrange("(r hh p) k -> hh p r k", r=N_CORES, hh=H)
            v_view = cc_out_v[:].rearrange("(kb p) (hh d) -> hh p kb d", p=128, hh=H)
            with (
                tc.tile_pool(name="kt_pool", bufs=3) as kt_pool,
                tc.tile_pool(name="vh_pool", bufs=3) as vh_pool,
                tc.tile_pool(name="e_pool", bufs=4) as e_pool,
                tc.tile_pool(name="sacc_pool", bufs=3) as sacc_pool,
                tc.tile_pool(name="rcp_pool", bufs=3) as rcp_pool,
                tc.tile_pool(name="ps_s", bufs=2, space="PSUM") as ps_s_pool,
                tc.tile_pool(name="ps_o", bufs=2, space="PSUM") as ps_o_pool,
                tc.tile_pool(name="ps_r", bufs=2, space="PSUM") as ps_r_pool,
            ):
                for h in range(H):
                    kt_h = kt_pool.tile([128, N_CORES, SKV], f16, tag="kt")
                    nc.sync.dma_start(out=kt_h[:], in_=kt_view[h])
                    v_h = vh_pool.tile([128, 16, 128], bf16, tag="vh")
                    nc.sync.dma_start(out=v_h[:], in_=v_view[h])
                    for t in range(QT):
                        nkb = KB_PER_T[t]
                        qs = qT_sb[:, h, t * 128:(t + 1) * 128]
                        oT_ps = ps_o_pool.tile([128, 128], f32, tag="oT")
                        sum_acc = sacc_pool.tile([128, 128], f32, tag="sacc")
                        for kb in range(nkb):
                            s_ps = ps_s_pool.tile([128, 128], f32, tag="sc")
                            nc.tensor.matmul(
                                s_ps[:],
                                kt_h[:, kb // 2, (kb % 2) * 128:(kb % 2) * 128 + 128],
                                qs,
                                start=True, stop=True,
                            )
                            e = e_pool.tile([128, 128], bf16, tag="e")
                            nc.scalar.activation(e[:], s_ps[:], mybir.ActivationFunctionType.Exp)
                            midx = kb if t == 0 else 8 + kb
                            nc.vector.tensor_mul(e[:], e[:], mask_sb[:, midx, :])
                            if kb == 0:
                                nc.vector.tensor_copy(sum_acc[:], e[:])
                            else:
                                nc.vector.tensor_add(sum_acc[:], sum_acc[:], e[:])
                            nc.tensor.matmul(
                                oT_ps[:], v_h[:, kb, :], e[:],
                                start=(kb == 0), stop=(kb == nkb - 1),
                            )
                        # partition-reduce the per-(h,t) exp sums, reciprocal,
                        # broadcast along partitions via rank-1 matmul
                        ssum_ps = ps_r_pool.tile([1, 128], f32, tag="ssum")
                        nc.tensor.matmul(ssum_ps[:], ones_sb[:], sum_acc[:], start=True, stop=True)
                        rcp = rcp_pool.tile([1, 128], f32, tag="rcp")
                        nc.vector.reciprocal(rcp[:], ssum_ps[:])
                        rb_ps = ps_r_pool.tile([128, 128], f32, tag="rb")
                        nc.tensor.matmul(rb_ps[:], ones_row[:], rcp[:], start=True, stop=True)
                        rb_sb = rcp_pool.tile([128, 128], f32, tag="rbs")
                        nc.vector.tensor_copy(rb_sb[:], rb_ps[:])
                        nc.vector.tensor_mul(attnT_sb[:, t, h, :], oT_ps[:], rb_sb[:])

            # ---- phase 5: output projection -------------------------------------
            with (
                tc.tile_pool(name="wo_pool", bufs=4) as wo_pool,
                tc.tile_pool(name="osb_pool", bufs=3) as osb_pool,
                tc.tile_pool(name="pf_psum", bufs=2, space="PSUM") as pf_psum,
            ):
                for ng in range(4):
                    ps_t = [pf_psum.tile([128, 512], f32, tag="pf", name=f"pf{ng}_{t}") for t in range(QT)]
                    for h in range(H):
                        wo_t = wo_pool.tile([128, 512], f16, tag="wo")
                        nc.sync.dma_start(
                            out=wo_t[:],
                            in_=woT_e[h * 128:(h + 1) * 128, ng * 512:(ng + 1) * 512],
                        )
                        for t in range(QT):
                            nc.tensor.matmul(
                                ps_t[t][:], attnT_sb[:, t, h, :], wo_t[:],
                                start=(h == 0), stop=(h == H - 1),
                            )
                    for t in range(QT):
                        osb = osb_pool.tile([128, 512], f32, tag="osb")
                        nc.vector.tensor_copy(osb[:], ps_t[t][:])
                        nc.sync.dma_start(
                            out=out_e[t * 128:(t + 1) * 128, ng * 512:(ng + 1) * 512],
                            in_=osb[:],
                        )

    _legalize_waits(nc)
    return nc


def _legalize_waits(nc):
    """walrus rejects instructions with too many sync waits (1 for pseudo-DMA,
    2 for most others); split excess waits onto preceding same-engine NOPs
    (engine streams are in-order, so a NOP-wait just before is equivalent)."""
    from concourse import mybir

    for bb in nc.main_func.blocks:
        insts = list(bb.instructions)
        out = []
        for ins in insts:
            max_waits = 1
            si = ins.sync_info
            if si is not None and si.on_wait and len(si.on_wait) > max_waits:
                waits = list(si.on_wait)
                excess, keep = waits[:-max_waits], waits[-max_waits:]
                for w in excess:
                    nop = nc.engines[ins.engine].nop(nofuse=True, hint="wait_split").ins
                    for bb2 in nc.main_func.blocks:
                        if nop in bb2.instructions:
                            bb2.instructions.remove(nop)
                            break
                    nop.sync_info = mybir.SyncInfo(on_wait=[w], on_update=[])
                    out.append(nop)
                si.on_wait = keep
            out.append(ins)
        bb.instructions[:] = out


def kernel(**inputs):
    global LAST_RESULT
    from concourse.bass_utils import run_bass_kernel_spmd

    x = np.asarray(inputs["x"], dtype=np.float32)           # (1, S, DIM)
    mask = np.asarray(inputs["mask"], dtype=np.float32)     # (1,1,S,S)
    sll = np.asarray(inputs["section_log_len"], dtype=np.float32)  # (1,1,S,1)
    wq = np.asarray(inputs["wq"], dtype=np.float32)
    wk = np.asarray(inputs["wk"], dtype=np.float32)
    wv = np.asarray(inputs["wv"], dtype=np.float32)
    wo = np.asarray(inputs["wo"], dtype=np.float32)
    ss = np.asarray(inputs["seq_scale"], dtype=np.float32)  # (1,H,1,1)

    x2 = x[0]                       # (S, DIM)
    m2 = mask[0, 0]                 # (S, S)
    sll1 = sll[0, 0, :, 0]          # (S,)
    ss1 = ss[0, :, 0, 0]            # (H,)

    f16 = np.float16
    bf16 = ml_dtypes.bfloat16

    # host-folded weights (shared across cores)
    wq_s = wq * (np.repeat(ss1, HD) / math.sqrt(HD))[:, None]
    wqT = np.ascontiguousarray(wq_s.T, dtype=f16)
    wkT = np.ascontiguousarray(wk.T, dtype=f16)
    wvT = np.ascontiguousarray(wv.T, dtype=f16)
    woT = np.ascontiguousarray(wo.T, dtype=f16)

    in_maps = []
    for i in range(N_CORES):
        qrows = np.arange(i, S, N_CORES)
        xq = x2[qrows] * sll1[qrows, None]
        xqT = np.ascontiguousarray(xq.T, dtype=f16)
        xkvT = np.ascontiguousarray(x2[i * SKV:(i + 1) * SKV].T, dtype=f16)
        # multiplicative masks: exp(additive mask), transposed to (k', r)
        qr = qrows.reshape(QT, 128)
        blocks = []
        for t in range(QT):
            nkb = KB_PER_T[t]
            sub = m2[qr[t]][:, :nkb * 128]              # (128 r, nkb*128 k)
            sub = np.exp(np.maximum(sub, -700.0))
            sub = sub.reshape(128, nkb, 128).transpose(1, 2, 0)   # (nkb, k', r)
            blocks.append(sub)
        maskM = np.ascontiguousarray(
            np.concatenate(blocks, axis=0), dtype=bf16)  # (24,128,128)
        in_maps.append({
            "xqT": xqT, "xkvT": xkvT, "maskM": maskM,
            "wqT": wqT, "wkT": wkT, "wvT": wvT, "woT": woT,
        })

    if "nc" not in _CACHE:
        _CACHE["nc"] = _build_graph()
    nc = _CACHE["nc"]

    res = run_bass_kernel_spmd(nc, in_maps, core_ids=list(range(N_CORES)))
    LAST_RESULT = res

    out = np.zeros((S, DIM), dtype=np.float32)
    for i in range(N_CORES):
        out[i::N_CORES] = np.asarray(res.results[i]["out"], dtype=np.float32)
    return out.reshape(1, S, DIM)
